# revision 20
# baseline (speedup 1.0000x reference)
"""TRN2 Bass kernel for nn_EMAModule (EM attention module).

See kernel_v1_full.py docstring for the full design notes. This version has
feature flags (EMA_FLAGS env) to bisect an HW-only hang:
  colsum: 4-matmul z colsums + s-fold        (else baseline s_ps/qinv path)
  gt:     GT-form M-step moment              (else baseline G + PE transposes)
  ttr:    fused sq+reduce (tensor_tensor_reduce)
  of16:   fp16 output DMA
  odma:   ZT ones row via DMA                (else memset)
  res:    half residual adds on DVE          (else all PE ident + copies)
  inpz:   exp writes Z in-place              (else separate E tile)
"""
import numpy as np
import os

import concourse.bacc as bacc
import concourse.bass as bass
import concourse.tile as tile
from concourse import mybir
from concourse import bass_utils
from concourse.masks import make_identity

F32 = mybir.dt.float32
F16 = mybir.dt.float16
F8 = mybir.dt.float8e4
AF = mybir.ActivationFunctionType
ALU = mybir.AluOpType
AX = mybir.AxisListType

B, C, H, W, K = 16, 512, 64, 64, 64
N = H * W
NCORES = 8
SPC = B // NCORES
T = 3
BN_EPS = 1e-5
EXP_SHIFT = -6.0
CC = C // 128
NT = N // 128
NQ = 4
NTQ = NT // NQ
NK = N // 512

# NOTE: tensor_tensor_reduce ("ttr") crashes the DVE exec unit on HW
# (NRT_EXEC_UNIT_UNRECOVERABLE status_code=101) despite passing CoreSim —
# do not re-enable without re-validating on hardware.
_F = os.environ.get("EMA_FLAGS", "colsum,gt,of16,odma,res,inpz").split(",")
USE_COLSUM = "colsum" in _F
USE_GT = "gt" in _F
USE_TTR = "ttr" in _F
OUT_F16 = "of16" in _F
ONES_DMA = "odma" in _F
SPLIT_RES = "res" in _F
INPLACE_Z = "inpz" in _F


def ts(i, sz):
    return bass.ts(i, sz)


PHASE_MARKS = []


def _mark(nc, label):
    PHASE_MARKS.append((label, nc.next_id()))


def build_bass():
    nc = bacc.Bacc("TRN2", target_bir_lowering=False, debug=False,
                   num_devices=NCORES)
    dram = lambda name, shape, dt, kind: nc.dram_tensor(name, shape, dt, kind=kind).ap()
    x8 = dram("x8", [SPC, 128, CC, N], F8, "ExternalInput")      # logits
    xres = dram("xres", [SPC, CC, 128, N], F16, "ExternalInput")  # residual
    xt8 = dram("xt8", [SPC, 128, NT, C], F8, "ExternalInput")     # M-step
    # wcat0: [m2t0 = w_in^T bases^T (host-folded iter-0 weight) | b_in]
    wcat0 = dram("wcat0", [128, CC, K + 1], F16, "ExternalInput")
    # wcat1: [w_in | w_in^T | (w_out*BN_inv)^T]
    wcat1 = dram("wcat1", [128, CC, 3 * C], F16, "ExternalInput")
    ebb0 = dram("ebb0", [128, K], F16, "ExternalInput")  # exp(b_in.bases^T) replicated
    srow16 = dram("srow16", [1, C], F16, "ExternalInput")
    binrow = dram("binrow", [1, C], F32, "ExternalInput")
    ones16 = dram("ones16", [1, N], F16, "ExternalInput")
    outp = dram("outp", [SPC, CC, 128, N], F16 if OUT_F16 else F32,
                "ExternalOutput")

    with tile.TileContext(nc) as tc:
        with (
            tc.tile_pool(name="const", bufs=1) as cpool,
            tc.tile_pool(name="xin", bufs=2) as xpool,
            tc.tile_pool(name="xt", bufs=2) as xtpool,
            tc.tile_pool(name="work", bufs=2) as wpool,
            tc.tile_pool(name="outsb", bufs=4) as opool,
            tc.tile_pool(name="lg", bufs=3, space="PSUM") as lgpool,
            tc.tile_pool(name="sc", bufs=1, space="PSUM") as scpool,
            tc.tile_pool(name="srow", bufs=2, space="PSUM") as rowpool,
            tc.tile_pool(name="ssum", bufs=1, space="PSUM") as spool,
        ):
            wcat0_sb = cpool.tile([128, CC, K + 1], F16)
            nc.sync.dma_start(out=wcat0_sb, in_=wcat0)
            m2t0_sb = wcat0_sb[:, :, 0:K]
            bin_sb = wcat0_sb[:, :, K:K + 1]
            wcat1_sb = cpool.tile([128, CC, 3 * C], F16)
            w_sb = wcat1_sb[:, :, 0:C]
            wt_sb = wcat1_sb[:, :, C:2 * C]
            wot_sb = wcat1_sb[:, :, 2 * C:3 * C]
            ebb0_sb = cpool.tile([128, K], F16)
            binb_sb = cpool.tile([K, C], F32)
            ident = cpool.tile([128, 128], F16)
            make_identity(nc, ident)
            ones_row = cpool.tile([1, 128], F16)
            nc.vector.memset(ones_row, 1.0)
            ones_col = cpool.tile([128, 1], F16)
            nc.vector.memset(ones_col, 1.0)
            expbias = cpool.tile([128, 1], F32)
            nc.vector.memset(expbias, EXP_SHIFT)
            ident32 = cpool.tile([1, 1], F32)
            nc.vector.memset(ident32, 1.0)

            # load order matters: the logits for BOTH samples consume X early,
            # while XT is first needed by the M-step (~10us later). Loading
            # X(s0), X(s1), XT(s0), XT(s1) keeps the PE fed through iter 0.
            X, XT, muT, Z = [None] * SPC, [None] * SPC, [None] * SPC, [None] * SPC
            for s in range(SPC):
                X[s] = xpool.tile([128, CC, N], F8, tag="x", name=f"X{s}")
                for q in range(NQ):
                    nc.sync.dma_start(out=X[s][:, :, ts(q, N // NQ)],
                                      in_=x8[s][:, :, ts(q, N // NQ)])
            nc.sync.dma_start(out=ebb0_sb, in_=ebb0)
            nc.sync.dma_start(out=binb_sb, in_=bass.AP(
                tensor=binrow.tensor, offset=binrow.offset,
                ap=[[0, K]] + binrow.ap[1:]))
            for s in range(SPC):
                XT[s] = xtpool.tile([128, NT, C], F8, tag="xt", name=f"XT{s}")
                for q in range(NQ):
                    nc.sync.dma_start(out=XT[s][:, ts(q, NTQ), :],
                                      in_=xt8[s][:, ts(q, NTQ), :])
                if s == 0:
                    nc.sync.dma_start(out=wcat1_sb, in_=wcat1)

            r = [None] * SPC
            rv = [None] * SPC
            eb_b = [None] * SPC
            m2t_sb = [None] * SPC
            s_ps = [None] * SPC
            binb_scaled = [None] * SPC
            mu16 = [None] * SPC
            qinv = [None] * SPC
            cs_pend = []

            def emit_colsum(s, q):
                nc.tensor.matmul(s_ps[s], ones_col, Z[s][:, ts(q, NTQ), :],
                                 start=(q == 0), stop=(q == NQ - 1))

            def emit_s_tail(s):
                s_sb = wpool.tile([1, K], F32, tag="s", name=f"s_sb{s}")
                nc.vector.reduce_sum(
                    s_sb,
                    bass.AP(tensor=s_ps[s].tensor, offset=s_ps[s].offset,
                            ap=[s_ps[s].ap[0], [1, K], [K, NTQ]]),
                    axis=AX.X)
                scol_ps = rowpool.tile([K, 1], F32, tag="row",
                                       name=f"scol_ps{s}")
                nc.tensor.transpose(scol_ps, s_sb, ident32[0:1, 0:1])
                scol_sb = wpool.tile([K, 1], F32, tag="scol", name=f"scol_sb{s}")
                nc.scalar.copy(scol_sb, scol_ps)
                binb_scaled[s] = wpool.tile([K, C], F32, tag="binbs",
                                            name=f"binbs{s}")
                nc.vector.tensor_scalar(binb_scaled[s], binb_sb, scol_sb, None,
                                        op0=ALU.mult)

            def drain_colsums(upto=None):
                while cs_pend and (upto is None or len(cs_pend) > upto):
                    sq = cs_pend.pop(0)
                    emit_colsum(*sq)
                    if sq[1] == NQ - 1:
                        emit_s_tail(sq[0])

            for it in range(T):
                for s in range(SPC):
                    _mark(nc, f'it{it}_Ahead_s{s}')
                    if it == 0:
                        # iter-0 folded weights are host-precomputed
                        m2t_sb[s] = m2t0_sb
                        eb_b[s] = ebb0_sb
                        continue
                    # A-head
                    m2t_ps = scpool.tile([128, CC, K], F32, tag=f"sc{s}",
                                         name=f"m2t_ps{s}")
                    for cc in range(CC):
                        for oc in range(CC):
                            nc.tensor.matmul(
                                m2t_ps[:, cc, :],
                                w_sb[:, oc, ts(cc, 128)],
                                muT[s][:, oc, :],
                                start=(oc == 0), stop=(oc == CC - 1))
                    beta_ps = rowpool.tile([1, K], F32, tag="row",
                                           name=f"beta_ps{s}")
                    for oc in range(CC):
                        nc.tensor.matmul(beta_ps, bin_sb[:, oc, :], muT[s][:, oc, :],
                                         start=(oc == 0), stop=(oc == CC - 1))
                    m2t_sb[s] = wpool.tile([128, CC, K], F16, tag="m2t",
                                           name=f"m2t_sb{s}")
                    nc.scalar.copy(m2t_sb[s], m2t_ps)
                    beta_sb = wpool.tile([1, K], F16, tag="beta",
                                         name=f"beta_sb{s}")
                    nc.scalar.copy(beta_sb, beta_ps)
                    eb_row = wpool.tile([1, K], F16, tag="eb_row",
                                        name=f"eb_row{s}")
                    nc.scalar.activation(eb_row, beta_sb, AF.Exp)
                    eb_ps = rowpool.tile([128, K], F32, tag="row",
                                         name=f"eb_ps{s}")
                    nc.tensor.matmul(eb_ps, ones_row, eb_row, start=True, stop=True)
                    eb_b[s] = wpool.tile([128, K], F16, tag="eb_b",
                                         name=f"eb_b{s}")
                    nc.scalar.copy(eb_b[s], eb_ps)

                for s in range(SPC):
                    _mark(nc, f'it{it}_Abody_s{s}')
                    # A-body
                    Z[s] = wpool.tile([128, NT, K], F16, tag="Z", name=f"Z_{s}")
                    r[s] = wpool.tile([128, NT], F32, tag="r", name=f"r{s}")
                    rv[s] = wpool.tile([128, NT], F32, tag="rinv", name=f"rinv{s}")
                    if USE_COLSUM:
                        s_ps[s] = spool.tile([1, NTQ, K], F32, tag="ssum",
                                             name=f"s_ps{s}")
                    E = None
                    if not INPLACE_Z:
                        E = wpool.tile([128, NT, K], F16, tag="E", name=f"E{s}")
                    for q in range(NQ):
                        _mark(nc, f'it{it}_lg_s{s}_q{q}')
                        lg = lgpool.tile([128, NTQ, K], F32, tag="lg",
                                         name=f"lg{s}_{q}")
                        for t8 in range(NTQ):
                            t = q * NTQ + t8
                            for cc in range(CC):
                                nc.tensor.matmul(
                                    lg[:, t8, :],
                                    X[s][:, cc, ts(t, 128)],
                                    m2t_sb[s][:, cc, :],
                                    start=(cc == 0), stop=(cc == CC - 1))
                        if USE_COLSUM:
                            drain_colsums(upto=1)
                        Zq = Z[s][:, ts(q, NTQ), :]
                        tgt = Zq if INPLACE_Z else E[:, ts(q, NTQ), :]
                        nc.scalar.activation(tgt, lg, AF.Exp,
                                             bias=expbias, scale=1.0)
                        nc.vector.tensor_tensor(
                            out=tgt, in0=tgt,
                            in1=bass.AP(tensor=eb_b[s].tensor,
                                        offset=eb_b[s].offset,
                                        ap=[eb_b[s].ap[0], [0, NTQ], [1, K]]),
                            op=ALU.mult)
                        nc.vector.reduce_sum(r[s][:, ts(q, NTQ)], tgt, axis=AX.X)
                        rq = rv[s][:, ts(q, NTQ)]
                        nc.vector.reciprocal(rq, r[s][:, ts(q, NTQ)])
                        nc.vector.tensor_tensor(
                            out=Zq, in0=tgt,
                            in1=bass.AP(tensor=rv[s].tensor,
                                        offset=rq.offset,
                                        ap=[rq.ap[0], rq.ap[1], [0, K]]),
                            op=ALU.mult)
                        if USE_COLSUM:
                            cs_pend.append((s, q))

                for s in range(SPC):
                    _mark(nc, f'it{it}_Bmain_s{s}')
                    # B-main
                    if USE_COLSUM:
                        drain_colsums(upto=0 if s == SPC - 1 else 1)
                    if USE_GT:
                        # cc-outer: PSUM accumulation groups must be
                        # contiguous per zero-region (interleaving the four
                        # cc groups in one bank corrupts partial sums — the
                        # start flag zeroes region-granular). Group 0 already
                        # consumes XT tiles incrementally via subtile deps.
                        GT_ps = scpool.tile([128, CC, K], F32, tag=f"sc{s}",
                                            name=f"GT_ps{s}")
                        for cc in range(CC):
                            for t in range(NT):
                                nc.tensor.matmul(GT_ps[:, cc, :],
                                                 XT[s][:, t, ts(cc, 128)],
                                                 Z[s][:, t, :],
                                                 start=(t == 0), stop=(t == NT - 1))
                        GT_sb = wpool.tile([128, CC, K], F16, tag="GT",
                                           name=f"GT_sb{s}")
                        nc.scalar.copy(GT_sb, GT_ps)
                    else:
                        G_ps = scpool.tile([K, C], F32, tag=f"sc{s}",
                                           name=f"G_ps{s}")
                        for t in range(NT):
                            nc.tensor.matmul(G_ps, Z[s][:, t, :], XT[s][:, t, :],
                                             start=(t == 0), stop=(t == NT - 1))
                        G_sb = wpool.tile([K, C], F16, tag="G", name=f"G_sb{s}")
                        nc.vector.tensor_copy(G_sb, G_ps)
                        GT_ps2 = scpool.tile([128, CC, K], F16, tag=f"sc{s}",
                                             name=f"GT_ps{s}")
                        for cc in range(CC):
                            nc.tensor.transpose(GT_ps2[:, cc, :],
                                                G_sb[:, ts(cc, 128)],
                                                ident[0:K, 0:K])
                        GT_sb = wpool.tile([128, CC, K], F16, tag="GT",
                                           name=f"GT_sb{s}")
                        nc.scalar.copy(GT_sb, GT_ps2)
                    if not USE_COLSUM:
                        s_ps[s] = rowpool.tile([1, K], F32, tag="row",
                                               name=f"sps{s}")
                        for t in range(NT):
                            nc.tensor.matmul(s_ps[s], ones_col, Z[s][:, t, :],
                                             start=(t == 0), stop=(t == NT - 1))
                        s_sb = wpool.tile([1, K], F32, tag="s", name=f"s_sb{s}")
                        nc.scalar.copy(s_sb, s_ps[s])
                        scol_ps = rowpool.tile([K, 1], F32, tag="row",
                                               name=f"scol_ps{s}")
                        nc.tensor.transpose(scol_ps, s_sb, ident32[0:1, 0:1])
                        qinv[s] = wpool.tile([K, 1], F32, tag="qinv",
                                             name=f"qinv{s}")
                        nc.vector.tensor_scalar(qinv[s], scol_ps, 1e-12, None,
                                                op0=ALU.add)
                        nc.vector.reciprocal(qinv[s], qinv[s])
                    mu_ps = scpool.tile([K, C], F32, tag=f"sc{s}", name=f"mu_ps{s}")
                    for cc in range(CC):
                        nc.tensor.matmul(mu_ps, GT_sb[:, cc, :], wt_sb[:, cc, :],
                                         start=(cc == 0), stop=(cc == CC - 1))
                    mu1 = wpool.tile([K, C], F32, tag="mu1", name=f"mu1_{s}")
                    if USE_COLSUM:
                        nc.vector.tensor_tensor(out=mu1, in0=mu_ps,
                                                in1=binb_scaled[s], op=ALU.add)
                    else:
                        nc.vector.tensor_scalar(mu1, mu_ps, qinv[s], None,
                                                op0=ALU.mult)
                        nc.vector.tensor_tensor(out=mu1, in0=mu1, in1=binb_sb,
                                                op=ALU.add)
                    n2 = wpool.tile([K, 1], F32, tag="n2", name=f"n2_{s}")
                    if USE_TTR:
                        sqd = wpool.tile([K, C], F16, tag="sq", name=f"sq{s}")
                        nc.vector.tensor_tensor_reduce(
                            out=sqd, in0=mu1, in1=mu1, scale=1.0, scalar=0.0,
                            op0=ALU.mult, op1=ALU.add, accum_out=n2)
                    else:
                        sqd = wpool.tile([K, C], F32, tag="sq", name=f"sq{s}")
                        nc.vector.tensor_tensor(out=sqd, in0=mu1, in1=mu1,
                                                op=ALU.mult)
                        nc.vector.reduce_sum(n2, sqd, axis=AX.X)
                    yy = wpool.tile([K, 1], F32, tag="yy", name=f"yy{s}")
                    ti = wpool.tile([K, 1], mybir.dt.int32, tag="ti",
                                    name=f"ti{s}")
                    nc.vector.tensor_scalar(ti, n2.bitcast(mybir.dt.int32), 1,
                                            None, op0=ALU.logical_shift_right)
                    nc.vector.tensor_scalar(ti, ti, -1, None,
                                            op0=ALU.bitwise_xor)
                    nc.vector.tensor_scalar(yy.bitcast(mybir.dt.int32), ti,
                                            0x5f3759df + 1, None,
                                            op0=ALU.add)
                    tb = wpool.tile([K, 1], F32, tag="tb", name=f"tb{s}")
                    for _ in range(2):
                        nc.vector.tensor_tensor(out=tb, in0=yy, in1=yy,
                                                op=ALU.mult)
                        nc.vector.tensor_tensor(out=tb, in0=tb, in1=n2,
                                                op=ALU.mult)
                        nc.vector.tensor_scalar(tb, tb, -0.5, 1.5,
                                                op0=ALU.mult, op1=ALU.add)
                        nc.vector.tensor_tensor(out=yy, in0=yy, in1=tb,
                                                op=ALU.mult)
                    mu16[s] = wpool.tile([K, C], F16, tag="mu16",
                                         name=f"mu16_{s}")
                    nc.vector.tensor_scalar(mu16[s], mu1, yy, None,
                                            op0=ALU.mult)

                for s in range(SPC):
                    _mark(nc, f'it{it}_Btail_s{s}')
                    # B-tail
                    muT_ps = scpool.tile([128, CC, K], F16, tag=f"sc{s}",
                                         name=f"muT_ps{s}")
                    for cc in range(CC):
                        nc.tensor.transpose(muT_ps[:, cc, :],
                                            mu16[s][:, ts(cc, 128)],
                                            ident[0:K, 0:K])
                    muT_new = wpool.tile([128, CC, K], F16, tag="muT",
                                         name=f"muT_new{s}")
                    nc.scalar.copy(muT_new, muT_ps)
                    muT[s] = muT_new

            # output path
            m3s, ZT = [None] * SPC, [None] * SPC
            for s in range(SPC):
                _mark(nc, f'out_zt_s{s}')
                m3_ps = scpool.tile([K, C], F32, tag=f"sc{s}", name=f"m3_ps{s}")
                for cc in range(CC):
                    nc.tensor.matmul(m3_ps, muT[s][:, cc, :], wot_sb[:, cc, :],
                                     start=(cc == 0), stop=(cc == CC - 1))
                m3s[s] = wpool.tile([K + 1, C], F16, tag="m3s", name=f"m3s{s}")
                nc.scalar.copy(m3s[s][0:K, :], m3_ps)
                nc.sync.dma_start(out=m3s[s][K:K + 1, :], in_=srow16)
                ZT[s] = wpool.tile([K + 1, N], F16, tag="ZT", name=f"ZT{s}")
                if ONES_DMA:
                    nc.sync.dma_start(out=ZT[s][K:K + 1, :], in_=ones16)
                else:
                    nc.vector.memset(ZT[s][K:K + 1, :], 1.0)
                for g in range(NT // 4):
                    zt_ps = rowpool.tile([K, 4, 128], F16, tag="row",
                                         name=f"zt_ps{s}_{g}")
                    for j in range(4):
                        nc.tensor.transpose(zt_ps[:, j, :], Z[s][:, g * 4 + j, :],
                                            ident)
                    dst = ZT[s][0:K, ts(g, 512)].rearrange("p (a b) -> p a b", a=4)
                    if g % 2 == 0:
                        nc.vector.tensor_copy(dst, zt_ps)
                    else:
                        nc.scalar.copy(dst, zt_ps)

            # out2: o2 PSUM tiles rotate through 5 banks (lg pool 3 + the two
            # freed sc banks); osb staged into one [128, N] tile per (s, oc)
            # so the output DMA is a single trigger per (s, oc). The residual
            # x (fp16) streams in late as per-(s,oc) chunks; adds run on DVE
            # (fused with the PSUM->SBUF move) or gpsimd (after an ACT copy),
            # keeping the PE free of identity matmuls.
            odt = F16 if OUT_F16 else F32
            ocnt = 0
            for s in range(SPC):
                _mark(nc, f'out2_s{s}')
                for oc in range(CC):
                    xrc = opool.tile([128, N], F16, tag="xres",
                                     name=f"xres{s}_{oc}", bufs=4)
                    nc.sync.dma_start(out=xrc, in_=xres[s, oc])
                    osb = opool.tile([128, N], odt, tag="osb",
                                     name=f"osb{s}_{oc}", bufs=2)
                    for nk in range(NK):
                        if ocnt % 5 < 3:
                            o2 = lgpool.tile([128, 512], F32, tag="lg",
                                             name=f"o2_{s}_{oc}_{nk}")
                        else:
                            o2 = scpool.tile([128, 512], F32,
                                             tag=f"sc{ocnt % 2}",
                                             name=f"o2_{s}_{oc}_{nk}")
                        ocnt += 1
                        xin = xrc[:, ts(nk, 512)]
                        dst = osb[:, ts(nk, 512)]
                        if not SPLIT_RES:
                            nc.tensor.matmul(o2, m3s[s][:, ts(oc, 128)],
                                             ZT[s][:, ts(nk, 512)],
                                             start=True, stop=False)
                            nc.tensor.matmul(o2, ident, xin,
                                             start=False, stop=True)
                            if (oc + nk) % 2 == 0:
                                nc.scalar.copy(dst, o2)
                            else:
                                nc.vector.tensor_copy(dst, o2)
                        else:
                            nc.tensor.matmul(o2, m3s[s][:, ts(oc, 128)],
                                             ZT[s][:, ts(nk, 512)],
                                             start=True, stop=True)
                            if (oc * NK + nk) % 8 < 3:
                                # ACT moves PSUM->SBUF, gpsimd adds x in SBUF
                                nc.scalar.copy(dst, o2)
                                nc.gpsimd.tensor_tensor(out=dst, in0=dst,
                                                        in1=xin, op=ALU.add)
                            else:
                                nc.vector.tensor_tensor(out=dst, in0=o2,
                                                        in1=xin, op=ALU.add)
                    nc.sync.dma_start(out=outp[s, oc], in_=osb)

    nc.compile()
    return nc


_NC_CACHE = None
_RUN_KWARGS: dict = {}
_LAST_RESULTS = None


def _get_nc():
    global _NC_CACHE
    if _NC_CACHE is None:
        _NC_CACHE = build_bass()
    return _NC_CACHE


def build_in_maps(x, w_in, b_in, w_out, b_out, gamma, beta, running_mean,
                  running_var, bases):
    x = np.asarray(x, np.float32)
    w_in = np.asarray(w_in, np.float32)
    b_in = np.asarray(b_in, np.float32)
    w_out = np.asarray(w_out, np.float32)
    b_out = np.asarray(b_out, np.float32)
    gamma = np.asarray(gamma, np.float32)
    beta = np.asarray(beta, np.float32)
    running_mean = np.asarray(running_mean, np.float32)
    running_var = np.asarray(running_var, np.float32)
    bases = np.asarray(bases, np.float32)

    inv = gamma / np.sqrt(running_var + BN_EPS)
    S = b_out * inv + beta - running_mean * inv
    wot = (w_out * inv[:, None]).T

    import ml_dtypes
    xr = x.reshape(B, C, N)
    x8 = np.ascontiguousarray(
        xr.reshape(B, CC, 128, N).transpose(0, 2, 1, 3)).astype(ml_dtypes.float8_e4m3)
    xresh = np.ascontiguousarray(xr.reshape(B, CC, 128, N)).astype(np.float16)
    xt8 = np.ascontiguousarray(
        xr.transpose(0, 2, 1).reshape(B, NT, 128, C).transpose(0, 2, 1, 3)
    ).astype(ml_dtypes.float8_e4m3)

    # iter-0 folded weights (mu0 = bases, unnormalized)
    m2t0 = (w_in.T.astype(np.float16).astype(np.float32)
            @ bases.T.astype(np.float16).astype(np.float32))      # (C, K)
    ebb0 = np.broadcast_to(
        np.exp(b_in @ bases.T.astype(np.float16).astype(np.float32))[None, :],
        (128, K))

    chunk = lambda a, f: a.reshape(CC, 128, f).transpose(1, 0, 2)
    wcat0 = np.ascontiguousarray(np.concatenate([
        chunk(np.ascontiguousarray(m2t0), K), chunk(b_in, 1),
    ], axis=2)).astype(np.float16)
    wcat1 = np.ascontiguousarray(np.concatenate([
        chunk(w_in, C),
        chunk(np.ascontiguousarray(w_in.T), C),
        chunk(np.ascontiguousarray(wot), C),
    ], axis=2)).astype(np.float16)
    srow16 = S.reshape(1, C).astype(np.float16)
    binrow = b_in.reshape(1, C).astype(np.float32)
    ones16 = np.ones((1, N), np.float16)
    ebb0 = np.ascontiguousarray(ebb0).astype(np.float16)

    in_maps = []
    for core in range(NCORES):
        sl = slice(core * SPC, (core + 1) * SPC)
        in_maps.append({
            "x8": x8[sl], "xres": xresh[sl], "xt8": xt8[sl],
            "wcat0": wcat0, "wcat1": wcat1, "ebb0": ebb0,
            "srow16": srow16, "binrow": binrow, "ones16": ones16,
        })
    return in_maps


def kernel(x, w_in, b_in, w_out, b_out, gamma, beta, running_mean, running_var,
           bases):
    in_maps = build_in_maps(x, w_in, b_in, w_out, b_out, gamma, beta,
                            running_mean, running_var, bases)
    nc = _get_nc()
    res = bass_utils.run_bass_kernel_spmd(nc, in_maps, core_ids=list(range(NCORES)),
                                          **_RUN_KWARGS)
    global _LAST_RESULTS
    _LAST_RESULTS = res
    out = np.empty((B, C, N), np.float32)
    for core in range(NCORES):
        o = res.results[core]["outp"]
        out[core * SPC:(core + 1) * SPC] = o.astype(np.float32).reshape(SPC, C, N)
    return out.reshape(B, C, H, W)


# revision 21
# speedup vs baseline: 1.0461x; 1.0461x over previous
"""TRN2 Bass kernel for nn_EMAModule (EM attention module).

See kernel_v1_full.py docstring for the full design notes. This version has
feature flags (EMA_FLAGS env) to bisect an HW-only hang:
  colsum: 4-matmul z colsums + s-fold        (else baseline s_ps/qinv path)
  gt:     GT-form M-step moment              (else baseline G + PE transposes)
  ttr:    fused sq+reduce (tensor_tensor_reduce)
  of16:   fp16 output DMA
  odma:   ZT ones row via DMA                (else memset)
  res:    half residual adds on DVE          (else all PE ident + copies)
  inpz:   exp writes Z in-place              (else separate E tile)
"""
import numpy as np
import os

import concourse.bacc as bacc
import concourse.bass as bass
import concourse.tile as tile
from concourse import mybir
from concourse import bass_utils
from concourse.masks import make_identity

F32 = mybir.dt.float32
F16 = mybir.dt.float16
F8 = mybir.dt.float8e4
AF = mybir.ActivationFunctionType
ALU = mybir.AluOpType
AX = mybir.AxisListType

B, C, H, W, K = 16, 512, 64, 64, 64
N = H * W
NCORES = 8
SPC = B // NCORES
T = 3
BN_EPS = 1e-5
EXP_SHIFT = -6.0
CC = C // 128
NT = N // 128
NQ = 4
NTQ = NT // NQ
NK = N // 512

# NOTE: tensor_tensor_reduce ("ttr") crashes the DVE exec unit on HW
# (NRT_EXEC_UNIT_UNRECOVERABLE status_code=101) despite passing CoreSim —
# do not re-enable without re-validating on hardware.
_F = os.environ.get("EMA_FLAGS", "colsum,gt,of16,odma,res,inpz").split(",")
USE_COLSUM = "colsum" in _F
USE_GT = "gt" in _F
USE_TTR = "ttr" in _F
OUT_F16 = "of16" in _F
ONES_DMA = "odma" in _F
SPLIT_RES = "res" in _F
INPLACE_Z = "inpz" in _F


def ts(i, sz):
    return bass.ts(i, sz)


PHASE_MARKS = []


def _mark(nc, label):
    PHASE_MARKS.append((label, nc.next_id()))


def build_bass():
    nc = bacc.Bacc("TRN2", target_bir_lowering=False, debug=False,
                   num_devices=NCORES)
    dram = lambda name, shape, dt, kind: nc.dram_tensor(name, shape, dt, kind=kind).ap()
    x8 = dram("x8", [SPC, 128, CC, N], F8, "ExternalInput")      # logits
    xres = dram("xres", [SPC, CC, 128, N], F16, "ExternalInput")  # residual
    xt8 = dram("xt8", [SPC, 128, NT, C], F8, "ExternalInput")     # M-step
    # wcat0: [m2t0 = w_in^T bases^T (host-folded iter-0 weight) | b_in]
    wcat0 = dram("wcat0", [128, CC, K + 1], F16, "ExternalInput")
    # wcat1: [w_in | w_in^T | (w_out*BN_inv)^T]
    wcat1 = dram("wcat1", [128, CC, 3 * C], F16, "ExternalInput")
    ebb0 = dram("ebb0", [128, K], F16, "ExternalInput")  # exp(b_in.bases^T) replicated
    srow16 = dram("srow16", [1, C], F16, "ExternalInput")
    binrow = dram("binrow", [1, C], F32, "ExternalInput")
    ones16 = dram("ones16", [1, N], F16, "ExternalInput")
    outp = dram("outp", [SPC, CC, 128, N], F16 if OUT_F16 else F32,
                "ExternalOutput")

    with tile.TileContext(nc) as tc:
        with (
            tc.tile_pool(name="const", bufs=1) as cpool,
            tc.tile_pool(name="xin", bufs=2) as xpool,
            tc.tile_pool(name="xt", bufs=2) as xtpool,
            tc.tile_pool(name="work", bufs=2) as wpool,
            tc.tile_pool(name="outsb", bufs=4) as opool,
            tc.tile_pool(name="lg", bufs=3, space="PSUM") as lgpool,
            tc.tile_pool(name="sc", bufs=1, space="PSUM") as scpool,
            tc.tile_pool(name="srow", bufs=2, space="PSUM") as rowpool,
            tc.tile_pool(name="ssum", bufs=1, space="PSUM") as spool,
        ):
            wcat0_sb = cpool.tile([128, CC, K + 1], F16)
            nc.sync.dma_start(out=wcat0_sb, in_=wcat0)
            m2t0_sb = wcat0_sb[:, :, 0:K]
            bin_sb = wcat0_sb[:, :, K:K + 1]
            wcat1_sb = cpool.tile([128, CC, 3 * C], F16)
            w_sb = wcat1_sb[:, :, 0:C]
            wt_sb = wcat1_sb[:, :, C:2 * C]
            wot_sb = wcat1_sb[:, :, 2 * C:3 * C]
            ebb0_sb = cpool.tile([128, K], F16)
            binb_sb = cpool.tile([K, C], F32)
            ident = cpool.tile([128, 128], F16)
            make_identity(nc, ident)
            ones_row = cpool.tile([1, 128], F16)
            nc.vector.memset(ones_row, 1.0)
            ones_col = cpool.tile([128, 1], F16)
            nc.vector.memset(ones_col, 1.0)
            expbias = cpool.tile([128, 1], F32)
            nc.vector.memset(expbias, EXP_SHIFT)
            ident32 = cpool.tile([1, 1], F32)
            nc.vector.memset(ident32, 1.0)

            # load order matters: the logits for BOTH samples consume X early,
            # while XT is first needed by the M-step (~10us later). Loading
            # X(s0), X(s1), XT(s0), XT(s1) keeps the PE fed through iter 0.
            X, XT, muT, Z = [None] * SPC, [None] * SPC, [None] * SPC, [None] * SPC
            for s in range(SPC):
                X[s] = xpool.tile([128, CC, N], F8, tag="x", name=f"X{s}")
                for q in range(NQ):
                    nc.sync.dma_start(out=X[s][:, :, ts(q, N // NQ)],
                                      in_=x8[s][:, :, ts(q, N // NQ)])
            nc.sync.dma_start(out=ebb0_sb, in_=ebb0)
            nc.sync.dma_start(out=binb_sb, in_=bass.AP(
                tensor=binrow.tensor, offset=binrow.offset,
                ap=[[0, K]] + binrow.ap[1:]))
            XR = [None] * SPC
            for s in range(SPC):
                XT[s] = xtpool.tile([128, NT, C], F8, tag="xt", name=f"XT{s}")
                for q in range(NQ):
                    nc.sync.dma_start(out=XT[s][:, ts(q, NTQ), :],
                                      in_=xt8[s][:, ts(q, NTQ), :])
                if s == 0:
                    nc.sync.dma_start(out=wcat1_sb, in_=wcat1)
            # prefetch the fp16 residual chunks during the EM phase (the DMA
            # engines are otherwise idle from ~50us) so the output phase only
            # carries the osb writes
            for s in range(SPC):
                XR[s] = opool.tile([128, CC, N], F16, tag="xres",
                                   name=f"XR{s}", bufs=2)
                for oc in range(CC):
                    nc.sync.dma_start(out=XR[s][:, oc, :], in_=xres[s, oc])

            r = [None] * SPC
            rv = [None] * SPC
            eb_b = [None] * SPC
            m2t_sb = [None] * SPC
            s_ps = [None] * SPC
            binb_scaled = [None] * SPC
            mu16 = [None] * SPC
            qinv = [None] * SPC
            cs_pend = []

            def emit_colsum(s, q):
                nc.tensor.matmul(s_ps[s], ones_col, Z[s][:, ts(q, NTQ), :],
                                 start=(q == 0), stop=(q == NQ - 1))

            def emit_s_tail(s):
                s_sb = wpool.tile([1, K], F32, tag="s", name=f"s_sb{s}")
                nc.vector.reduce_sum(
                    s_sb,
                    bass.AP(tensor=s_ps[s].tensor, offset=s_ps[s].offset,
                            ap=[s_ps[s].ap[0], [1, K], [K, NTQ]]),
                    axis=AX.X)
                scol_ps = rowpool.tile([K, 1], F32, tag="row",
                                       name=f"scol_ps{s}")
                nc.tensor.transpose(scol_ps, s_sb, ident32[0:1, 0:1])
                scol_sb = wpool.tile([K, 1], F32, tag="scol", name=f"scol_sb{s}")
                nc.scalar.copy(scol_sb, scol_ps)
                binb_scaled[s] = wpool.tile([K, C], F32, tag="binbs",
                                            name=f"binbs{s}")
                nc.vector.tensor_scalar(binb_scaled[s], binb_sb, scol_sb, None,
                                        op0=ALU.mult)

            def drain_colsums(upto=None):
                while cs_pend and (upto is None or len(cs_pend) > upto):
                    sq = cs_pend.pop(0)
                    emit_colsum(*sq)
                    if sq[1] == NQ - 1:
                        emit_s_tail(sq[0])

            for it in range(T):
                for s in range(SPC):
                    _mark(nc, f'it{it}_Ahead_s{s}')
                    if it == 0:
                        # iter-0 folded weights are host-precomputed
                        m2t_sb[s] = m2t0_sb
                        eb_b[s] = ebb0_sb
                        continue
                    # A-head
                    m2t_ps = scpool.tile([128, CC, K], F32, tag=f"sc{s}",
                                         name=f"m2t_ps{s}")
                    for cc in range(CC):
                        for oc in range(CC):
                            nc.tensor.matmul(
                                m2t_ps[:, cc, :],
                                w_sb[:, oc, ts(cc, 128)],
                                muT[s][:, oc, :],
                                start=(oc == 0), stop=(oc == CC - 1))
                    beta_ps = rowpool.tile([1, K], F32, tag="row",
                                           name=f"beta_ps{s}")
                    for oc in range(CC):
                        nc.tensor.matmul(beta_ps, bin_sb[:, oc, :], muT[s][:, oc, :],
                                         start=(oc == 0), stop=(oc == CC - 1))
                    m2t_sb[s] = wpool.tile([128, CC, K], F16, tag="m2t",
                                           name=f"m2t_sb{s}")
                    nc.scalar.copy(m2t_sb[s], m2t_ps)
                    beta_sb = wpool.tile([1, K], F16, tag="beta",
                                         name=f"beta_sb{s}")
                    nc.scalar.copy(beta_sb, beta_ps)
                    eb_row = wpool.tile([1, K], F16, tag="eb_row",
                                        name=f"eb_row{s}")
                    nc.scalar.activation(eb_row, beta_sb, AF.Exp)
                    eb_ps = rowpool.tile([128, K], F32, tag="row",
                                         name=f"eb_ps{s}")
                    nc.tensor.matmul(eb_ps, ones_row, eb_row, start=True, stop=True)
                    eb_b[s] = wpool.tile([128, K], F16, tag="eb_b",
                                         name=f"eb_b{s}")
                    nc.scalar.copy(eb_b[s], eb_ps)

                for s in range(SPC):
                    _mark(nc, f'it{it}_Abody_s{s}')
                    # A-body
                    Z[s] = wpool.tile([128, NT, K], F16, tag="Z", name=f"Z_{s}")
                    r[s] = wpool.tile([128, NT], F32, tag="r", name=f"r{s}")
                    rv[s] = wpool.tile([128, NT], F32, tag="rinv", name=f"rinv{s}")
                    if USE_COLSUM:
                        s_ps[s] = spool.tile([1, NTQ, K], F32, tag="ssum",
                                             name=f"s_ps{s}")
                    E = None
                    if not INPLACE_Z:
                        E = wpool.tile([128, NT, K], F16, tag="E", name=f"E{s}")
                    for q in range(NQ):
                        _mark(nc, f'it{it}_lg_s{s}_q{q}')
                        lg = lgpool.tile([128, NTQ, K], F32, tag="lg",
                                         name=f"lg{s}_{q}")
                        for t8 in range(NTQ):
                            t = q * NTQ + t8
                            for cc in range(CC):
                                nc.tensor.matmul(
                                    lg[:, t8, :],
                                    X[s][:, cc, ts(t, 128)],
                                    m2t_sb[s][:, cc, :],
                                    start=(cc == 0), stop=(cc == CC - 1))
                        if USE_COLSUM:
                            drain_colsums(upto=1)
                        Zq = Z[s][:, ts(q, NTQ), :]
                        tgt = Zq if INPLACE_Z else E[:, ts(q, NTQ), :]
                        nc.scalar.activation(tgt, lg, AF.Exp,
                                             bias=expbias, scale=1.0)
                        nc.vector.tensor_tensor(
                            out=tgt, in0=tgt,
                            in1=bass.AP(tensor=eb_b[s].tensor,
                                        offset=eb_b[s].offset,
                                        ap=[eb_b[s].ap[0], [0, NTQ], [1, K]]),
                            op=ALU.mult)
                        nc.vector.reduce_sum(r[s][:, ts(q, NTQ)], tgt, axis=AX.X)
                        rq = rv[s][:, ts(q, NTQ)]
                        nc.vector.reciprocal(rq, r[s][:, ts(q, NTQ)])
                        nc.vector.tensor_tensor(
                            out=Zq, in0=tgt,
                            in1=bass.AP(tensor=rv[s].tensor,
                                        offset=rq.offset,
                                        ap=[rq.ap[0], rq.ap[1], [0, K]]),
                            op=ALU.mult)
                        if USE_COLSUM:
                            cs_pend.append((s, q))

                for s in range(SPC):
                    _mark(nc, f'it{it}_Bmain_s{s}')
                    # B-main
                    if USE_COLSUM:
                        drain_colsums(upto=0 if s == SPC - 1 else 1)
                    if USE_GT:
                        # cc-outer: PSUM accumulation groups must be
                        # contiguous per zero-region (interleaving the four
                        # cc groups in one bank corrupts partial sums — the
                        # start flag zeroes region-granular). Group 0 already
                        # consumes XT tiles incrementally via subtile deps.
                        GT_ps = scpool.tile([128, CC, K], F32, tag=f"sc{s}",
                                            name=f"GT_ps{s}")
                        for cc in range(CC):
                            for t in range(NT):
                                nc.tensor.matmul(GT_ps[:, cc, :],
                                                 XT[s][:, t, ts(cc, 128)],
                                                 Z[s][:, t, :],
                                                 start=(t == 0), stop=(t == NT - 1))
                        GT_sb = wpool.tile([128, CC, K], F16, tag="GT",
                                           name=f"GT_sb{s}")
                        nc.scalar.copy(GT_sb, GT_ps)
                    else:
                        G_ps = scpool.tile([K, C], F32, tag=f"sc{s}",
                                           name=f"G_ps{s}")
                        for t in range(NT):
                            nc.tensor.matmul(G_ps, Z[s][:, t, :], XT[s][:, t, :],
                                             start=(t == 0), stop=(t == NT - 1))
                        G_sb = wpool.tile([K, C], F16, tag="G", name=f"G_sb{s}")
                        nc.vector.tensor_copy(G_sb, G_ps)
                        GT_ps2 = scpool.tile([128, CC, K], F16, tag=f"sc{s}",
                                             name=f"GT_ps{s}")
                        for cc in range(CC):
                            nc.tensor.transpose(GT_ps2[:, cc, :],
                                                G_sb[:, ts(cc, 128)],
                                                ident[0:K, 0:K])
                        GT_sb = wpool.tile([128, CC, K], F16, tag="GT",
                                           name=f"GT_sb{s}")
                        nc.scalar.copy(GT_sb, GT_ps2)
                    if not USE_COLSUM:
                        s_ps[s] = rowpool.tile([1, K], F32, tag="row",
                                               name=f"sps{s}")
                        for t in range(NT):
                            nc.tensor.matmul(s_ps[s], ones_col, Z[s][:, t, :],
                                             start=(t == 0), stop=(t == NT - 1))
                        s_sb = wpool.tile([1, K], F32, tag="s", name=f"s_sb{s}")
                        nc.scalar.copy(s_sb, s_ps[s])
                        scol_ps = rowpool.tile([K, 1], F32, tag="row",
                                               name=f"scol_ps{s}")
                        nc.tensor.transpose(scol_ps, s_sb, ident32[0:1, 0:1])
                        qinv[s] = wpool.tile([K, 1], F32, tag="qinv",
                                             name=f"qinv{s}")
                        nc.vector.tensor_scalar(qinv[s], scol_ps, 1e-12, None,
                                                op0=ALU.add)
                        nc.vector.reciprocal(qinv[s], qinv[s])
                    mu_ps = scpool.tile([K, C], F32, tag=f"sc{s}", name=f"mu_ps{s}")
                    for cc in range(CC):
                        nc.tensor.matmul(mu_ps, GT_sb[:, cc, :], wt_sb[:, cc, :],
                                         start=(cc == 0), stop=(cc == CC - 1))
                    mu1 = wpool.tile([K, C], F32, tag="mu1", name=f"mu1_{s}")
                    if USE_COLSUM:
                        nc.vector.tensor_tensor(out=mu1, in0=mu_ps,
                                                in1=binb_scaled[s], op=ALU.add)
                    else:
                        nc.vector.tensor_scalar(mu1, mu_ps, qinv[s], None,
                                                op0=ALU.mult)
                        nc.vector.tensor_tensor(out=mu1, in0=mu1, in1=binb_sb,
                                                op=ALU.add)
                    n2 = wpool.tile([K, 1], F32, tag="n2", name=f"n2_{s}")
                    if USE_TTR:
                        sqd = wpool.tile([K, C], F16, tag="sq", name=f"sq{s}")
                        nc.vector.tensor_tensor_reduce(
                            out=sqd, in0=mu1, in1=mu1, scale=1.0, scalar=0.0,
                            op0=ALU.mult, op1=ALU.add, accum_out=n2)
                    else:
                        sqd = wpool.tile([K, C], F32, tag="sq", name=f"sq{s}")
                        nc.vector.tensor_tensor(out=sqd, in0=mu1, in1=mu1,
                                                op=ALU.mult)
                        nc.vector.reduce_sum(n2, sqd, axis=AX.X)
                    yy = wpool.tile([K, 1], F32, tag="yy", name=f"yy{s}")
                    ti = wpool.tile([K, 1], mybir.dt.int32, tag="ti",
                                    name=f"ti{s}")
                    nc.vector.tensor_scalar(ti, n2.bitcast(mybir.dt.int32), 1,
                                            None, op0=ALU.logical_shift_right)
                    nc.vector.tensor_scalar(ti, ti, -1, None,
                                            op0=ALU.bitwise_xor)
                    nc.vector.tensor_scalar(yy.bitcast(mybir.dt.int32), ti,
                                            0x5f3759df + 1, None,
                                            op0=ALU.add)
                    tb = wpool.tile([K, 1], F32, tag="tb", name=f"tb{s}")
                    for _ in range(2):
                        nc.vector.tensor_tensor(out=tb, in0=yy, in1=yy,
                                                op=ALU.mult)
                        nc.vector.tensor_tensor(out=tb, in0=tb, in1=n2,
                                                op=ALU.mult)
                        nc.vector.tensor_scalar(tb, tb, -0.5, 1.5,
                                                op0=ALU.mult, op1=ALU.add)
                        nc.vector.tensor_tensor(out=yy, in0=yy, in1=tb,
                                                op=ALU.mult)
                    mu16[s] = wpool.tile([K, C], F16, tag="mu16",
                                         name=f"mu16_{s}")
                    nc.vector.tensor_scalar(mu16[s], mu1, yy, None,
                                            op0=ALU.mult)

                for s in range(SPC):
                    _mark(nc, f'it{it}_Btail_s{s}')
                    # B-tail
                    muT_ps = scpool.tile([128, CC, K], F16, tag=f"sc{s}",
                                         name=f"muT_ps{s}")
                    for cc in range(CC):
                        nc.tensor.transpose(muT_ps[:, cc, :],
                                            mu16[s][:, ts(cc, 128)],
                                            ident[0:K, 0:K])
                    muT_new = wpool.tile([128, CC, K], F16, tag="muT",
                                         name=f"muT_new{s}")
                    nc.scalar.copy(muT_new, muT_ps)
                    muT[s] = muT_new

            # output path
            m3s, ZT = [None] * SPC, [None] * SPC
            for s in range(SPC):
                _mark(nc, f'out_zt_s{s}')
                m3_ps = scpool.tile([K, C], F32, tag=f"sc{s}", name=f"m3_ps{s}")
                for cc in range(CC):
                    nc.tensor.matmul(m3_ps, muT[s][:, cc, :], wot_sb[:, cc, :],
                                     start=(cc == 0), stop=(cc == CC - 1))
                m3s[s] = wpool.tile([K + 1, C], F16, tag="m3s", name=f"m3s{s}")
                nc.scalar.copy(m3s[s][0:K, :], m3_ps)
                nc.sync.dma_start(out=m3s[s][K:K + 1, :], in_=srow16)
                ZT[s] = wpool.tile([K + 1, N], F16, tag="ZT", name=f"ZT{s}")
                if ONES_DMA:
                    nc.sync.dma_start(out=ZT[s][K:K + 1, :], in_=ones16)
                else:
                    nc.vector.memset(ZT[s][K:K + 1, :], 1.0)
                for g in range(NT // 4):
                    zt_ps = rowpool.tile([K, 4, 128], F16, tag="row",
                                         name=f"zt_ps{s}_{g}")
                    for j in range(4):
                        nc.tensor.transpose(zt_ps[:, j, :], Z[s][:, g * 4 + j, :],
                                            ident)
                    dst = ZT[s][0:K, ts(g, 512)].rearrange("p (a b) -> p a b", a=4)
                    if g % 2 == 0:
                        nc.vector.tensor_copy(dst, zt_ps)
                    else:
                        nc.scalar.copy(dst, zt_ps)

            # out2: o2 PSUM tiles rotate through 5 banks (lg pool 3 + the two
            # freed sc banks); osb staged into one [128, N] tile per (s, oc)
            # so the output DMA is a single trigger per (s, oc). The residual
            # x (fp16) streams in late as per-(s,oc) chunks; adds run on DVE
            # (fused with the PSUM->SBUF move) or gpsimd (after an ACT copy),
            # keeping the PE free of identity matmuls.
            odt = F16 if OUT_F16 else F32
            ocnt = 0
            for s in range(SPC):
                _mark(nc, f'out2_s{s}')
                for oc in range(CC):
                    xrc = XR[s][:, oc, :]
                    osb = opool.tile([128, N], odt, tag="osb",
                                     name=f"osb{s}_{oc}", bufs=2)
                    for nk in range(NK):
                        if ocnt % 5 < 3:
                            o2 = lgpool.tile([128, 512], F32, tag="lg",
                                             name=f"o2_{s}_{oc}_{nk}")
                        else:
                            o2 = scpool.tile([128, 512], F32,
                                             tag=f"sc{ocnt % 2}",
                                             name=f"o2_{s}_{oc}_{nk}")
                        ocnt += 1
                        xin = xrc[:, ts(nk, 512)]
                        dst = osb[:, ts(nk, 512)]
                        if not SPLIT_RES:
                            nc.tensor.matmul(o2, m3s[s][:, ts(oc, 128)],
                                             ZT[s][:, ts(nk, 512)],
                                             start=True, stop=False)
                            nc.tensor.matmul(o2, ident, xin,
                                             start=False, stop=True)
                            if (oc + nk) % 2 == 0:
                                nc.scalar.copy(dst, o2)
                            else:
                                nc.vector.tensor_copy(dst, o2)
                        else:
                            nc.tensor.matmul(o2, m3s[s][:, ts(oc, 128)],
                                             ZT[s][:, ts(nk, 512)],
                                             start=True, stop=True)
                            if (oc * NK + nk) % 8 < 3:
                                # ACT moves PSUM->SBUF, gpsimd adds x in SBUF
                                nc.scalar.copy(dst, o2)
                                nc.gpsimd.tensor_tensor(out=dst, in0=dst,
                                                        in1=xin, op=ALU.add)
                            else:
                                nc.vector.tensor_tensor(out=dst, in0=o2,
                                                        in1=xin, op=ALU.add)
                    nc.sync.dma_start(out=outp[s, oc], in_=osb)

    nc.compile()
    return nc


_NC_CACHE = None
_RUN_KWARGS: dict = {}
_LAST_RESULTS = None


def _get_nc():
    global _NC_CACHE
    if _NC_CACHE is None:
        _NC_CACHE = build_bass()
    return _NC_CACHE


def build_in_maps(x, w_in, b_in, w_out, b_out, gamma, beta, running_mean,
                  running_var, bases):
    x = np.asarray(x, np.float32)
    w_in = np.asarray(w_in, np.float32)
    b_in = np.asarray(b_in, np.float32)
    w_out = np.asarray(w_out, np.float32)
    b_out = np.asarray(b_out, np.float32)
    gamma = np.asarray(gamma, np.float32)
    beta = np.asarray(beta, np.float32)
    running_mean = np.asarray(running_mean, np.float32)
    running_var = np.asarray(running_var, np.float32)
    bases = np.asarray(bases, np.float32)

    inv = gamma / np.sqrt(running_var + BN_EPS)
    S = b_out * inv + beta - running_mean * inv
    wot = (w_out * inv[:, None]).T

    import ml_dtypes
    xr = x.reshape(B, C, N)
    x8 = np.ascontiguousarray(
        xr.reshape(B, CC, 128, N).transpose(0, 2, 1, 3)).astype(ml_dtypes.float8_e4m3)
    xresh = np.ascontiguousarray(xr.reshape(B, CC, 128, N)).astype(np.float16)
    xt8 = np.ascontiguousarray(
        xr.transpose(0, 2, 1).reshape(B, NT, 128, C).transpose(0, 2, 1, 3)
    ).astype(ml_dtypes.float8_e4m3)

    # iter-0 folded weights (mu0 = bases, unnormalized)
    m2t0 = (w_in.T.astype(np.float16).astype(np.float32)
            @ bases.T.astype(np.float16).astype(np.float32))      # (C, K)
    ebb0 = np.broadcast_to(
        np.exp(b_in @ bases.T.astype(np.float16).astype(np.float32))[None, :],
        (128, K))

    chunk = lambda a, f: a.reshape(CC, 128, f).transpose(1, 0, 2)
    wcat0 = np.ascontiguousarray(np.concatenate([
        chunk(np.ascontiguousarray(m2t0), K), chunk(b_in, 1),
    ], axis=2)).astype(np.float16)
    wcat1 = np.ascontiguousarray(np.concatenate([
        chunk(w_in, C),
        chunk(np.ascontiguousarray(w_in.T), C),
        chunk(np.ascontiguousarray(wot), C),
    ], axis=2)).astype(np.float16)
    srow16 = S.reshape(1, C).astype(np.float16)
    binrow = b_in.reshape(1, C).astype(np.float32)
    ones16 = np.ones((1, N), np.float16)
    ebb0 = np.ascontiguousarray(ebb0).astype(np.float16)

    in_maps = []
    for core in range(NCORES):
        sl = slice(core * SPC, (core + 1) * SPC)
        in_maps.append({
            "x8": x8[sl], "xres": xresh[sl], "xt8": xt8[sl],
            "wcat0": wcat0, "wcat1": wcat1, "ebb0": ebb0,
            "srow16": srow16, "binrow": binrow, "ones16": ones16,
        })
    return in_maps


def kernel(x, w_in, b_in, w_out, b_out, gamma, beta, running_mean, running_var,
           bases):
    in_maps = build_in_maps(x, w_in, b_in, w_out, b_out, gamma, beta,
                            running_mean, running_var, bases)
    nc = _get_nc()
    res = bass_utils.run_bass_kernel_spmd(nc, in_maps, core_ids=list(range(NCORES)),
                                          **_RUN_KWARGS)
    global _LAST_RESULTS
    _LAST_RESULTS = res
    out = np.empty((B, C, N), np.float32)
    for core in range(NCORES):
        o = res.results[core]["outp"]
        out[core * SPC:(core + 1) * SPC] = o.astype(np.float32).reshape(SPC, C, N)
    return out.reshape(B, C, H, W)


# revision 23
# speedup vs baseline: 1.0590x; 1.0123x over previous
"""TRN2 Bass kernel for nn_EMAModule (EM attention module).

See kernel_v1_full.py docstring for the full design notes. This version has
feature flags (EMA_FLAGS env) to bisect an HW-only hang:
  colsum: 4-matmul z colsums + s-fold        (else baseline s_ps/qinv path)
  gt:     GT-form M-step moment              (else baseline G + PE transposes)
  ttr:    fused sq+reduce (tensor_tensor_reduce)
  of16:   fp16 output DMA
  odma:   ZT ones row via DMA                (else memset)
  res:    half residual adds on DVE          (else all PE ident + copies)
  inpz:   exp writes Z in-place              (else separate E tile)
"""
import numpy as np
import os

import concourse.bacc as bacc
import concourse.bass as bass
import concourse.tile as tile
from concourse import mybir
from concourse import bass_utils
from concourse.masks import make_identity

F32 = mybir.dt.float32
F16 = mybir.dt.float16
F8 = mybir.dt.float8e4
AF = mybir.ActivationFunctionType
ALU = mybir.AluOpType
AX = mybir.AxisListType

B, C, H, W, K = 16, 512, 64, 64, 64
N = H * W
NCORES = 8
SPC = B // NCORES
T = 3
BN_EPS = 1e-5
EXP_SHIFT = -6.0
CC = C // 128
NT = N // 128
NQ = 4
NTQ = NT // NQ
NK = N // 512

# NOTE: tensor_tensor_reduce ("ttr") crashes the DVE exec unit on HW
# (NRT_EXEC_UNIT_UNRECOVERABLE status_code=101) despite passing CoreSim —
# do not re-enable without re-validating on hardware.
_F = os.environ.get("EMA_FLAGS", "colsum,gt,of16,odma,res,inpz").split(",")
USE_COLSUM = "colsum" in _F
USE_GT = "gt" in _F
USE_TTR = "ttr" in _F
OUT_F16 = "of16" in _F
ONES_DMA = "odma" in _F
SPLIT_RES = "res" in _F
INPLACE_Z = "inpz" in _F


def ts(i, sz):
    return bass.ts(i, sz)


PHASE_MARKS = []


def _mark(nc, label):
    PHASE_MARKS.append((label, nc.next_id()))


def build_bass():
    nc = bacc.Bacc("TRN2", target_bir_lowering=False, debug=False,
                   num_devices=NCORES)
    dram = lambda name, shape, dt, kind: nc.dram_tensor(name, shape, dt, kind=kind).ap()
    x8 = dram("x8", [SPC, 128, CC, N], F8, "ExternalInput")      # logits
    xres = dram("xres", [SPC, CC, 128, N], F16, "ExternalInput")  # residual
    xt8 = dram("xt8", [SPC, 128, NT, C], F8, "ExternalInput")     # M-step
    # wcat0: [m2t0 = w_in^T bases^T (host-folded iter-0 weight) | b_in]
    wcat0 = dram("wcat0", [128, CC, K + 1], F16, "ExternalInput")
    # wcat1: [w_in | w_in^T | (w_out*BN_inv)^T]
    wcat1 = dram("wcat1", [128, CC, 3 * C], F16, "ExternalInput")
    ebb0 = dram("ebb0", [128, K], F16, "ExternalInput")  # exp(b_in.bases^T) replicated
    srow16 = dram("srow16", [1, C], F16, "ExternalInput")
    binrow = dram("binrow", [1, C], F32, "ExternalInput")
    ones16 = dram("ones16", [1, N], F16, "ExternalInput")
    outp = dram("outp", [SPC, CC, 128, N], F16 if OUT_F16 else F32,
                "ExternalOutput")

    with tile.TileContext(nc) as tc:
        with (
            tc.tile_pool(name="const", bufs=1) as cpool,
            tc.tile_pool(name="xin", bufs=2) as xpool,
            tc.tile_pool(name="xt", bufs=2) as xtpool,
            tc.tile_pool(name="work", bufs=2) as wpool,
            tc.tile_pool(name="outsb", bufs=4) as opool,
            tc.tile_pool(name="lg", bufs=3, space="PSUM") as lgpool,
            tc.tile_pool(name="sc", bufs=1, space="PSUM") as scpool,
            tc.tile_pool(name="srow", bufs=2, space="PSUM") as rowpool,
            tc.tile_pool(name="ssum", bufs=1, space="PSUM") as spool,
        ):
            wcat0_sb = cpool.tile([128, CC, K + 1], F16)
            nc.sync.dma_start(out=wcat0_sb, in_=wcat0)
            m2t0_sb = wcat0_sb[:, :, 0:K]
            bin_sb = wcat0_sb[:, :, K:K + 1]
            wcat1_sb = cpool.tile([128, CC, 3 * C], F16)
            w_sb = wcat1_sb[:, :, 0:C]
            wt_sb = wcat1_sb[:, :, C:2 * C]
            wot_sb = wcat1_sb[:, :, 2 * C:3 * C]
            ebb0_sb = cpool.tile([128, K], F16)
            binb_sb = cpool.tile([K, C], F32)
            ident = cpool.tile([128, 128], F16)
            make_identity(nc, ident)
            ones_row = cpool.tile([1, 128], F16)
            nc.vector.memset(ones_row, 1.0)
            ones_col = cpool.tile([128, 1], F16)
            nc.vector.memset(ones_col, 1.0)
            expbias = cpool.tile([128, 1], F32)
            nc.vector.memset(expbias, EXP_SHIFT)
            ident32 = cpool.tile([1, 1], F32)
            nc.vector.memset(ident32, 1.0)

            # load order matters: the logits for BOTH samples consume X early,
            # while XT is first needed by the M-step (~10us later). Loading
            # X(s0), X(s1), XT(s0), XT(s1) keeps the PE fed through iter 0.
            X, XT, muT, Z = [None] * SPC, [None] * SPC, [None] * SPC, [None] * SPC
            for s in range(SPC):
                X[s] = xpool.tile([128, CC, N], F8, tag="x", name=f"X{s}")
                for q in range(NQ):
                    nc.sync.dma_start(out=X[s][:, :, ts(q, N // NQ)],
                                      in_=x8[s][:, :, ts(q, N // NQ)])
            nc.sync.dma_start(out=ebb0_sb, in_=ebb0)
            nc.sync.dma_start(out=binb_sb, in_=bass.AP(
                tensor=binrow.tensor, offset=binrow.offset,
                ap=[[0, K]] + binrow.ap[1:]))
            XR = [None] * SPC
            for s in range(SPC):
                XT[s] = xtpool.tile([128, NT, C], F8, tag="xt", name=f"XT{s}")
                for q in range(NQ):
                    nc.sync.dma_start(out=XT[s][:, ts(q, NTQ), :],
                                      in_=xt8[s][:, ts(q, NTQ), :])
                if s == 0:
                    nc.sync.dma_start(out=wcat1_sb, in_=wcat1)
            # prefetch the fp16 residual chunks during the EM phase (the DMA
            # engines are otherwise idle from ~50us) so the output phase only
            # carries the osb writes
            for s in range(SPC):
                XR[s] = opool.tile([128, CC, N], F16, tag="xres",
                                   name=f"XR{s}", bufs=2)
                for oc in range(CC):
                    nc.sync.dma_start(out=XR[s][:, oc, :], in_=xres[s, oc])

            r = [None] * SPC
            rv = [None] * SPC
            eb_b = [None] * SPC
            m2t_sb = [None] * SPC
            s_ps = [None] * SPC
            binb_scaled = [None] * SPC
            mu16 = [None] * SPC
            qinv = [None] * SPC
            cs_pend = []

            def emit_colsum(s, q):
                nc.tensor.matmul(s_ps[s], ones_col, Z[s][:, ts(q, NTQ), :],
                                 start=(q == 0), stop=(q == NQ - 1))

            def emit_s_tail(s):
                s_sb = wpool.tile([1, K], F32, tag="s", name=f"s_sb{s}")
                nc.vector.reduce_sum(
                    s_sb,
                    bass.AP(tensor=s_ps[s].tensor, offset=s_ps[s].offset,
                            ap=[s_ps[s].ap[0], [1, K], [K, NTQ]]),
                    axis=AX.X)
                scol_ps = rowpool.tile([K, 1], F32, tag="row",
                                       name=f"scol_ps{s}")
                nc.tensor.transpose(scol_ps, s_sb, ident32[0:1, 0:1])
                scol_sb = wpool.tile([K, 1], F32, tag="scol", name=f"scol_sb{s}")
                nc.scalar.copy(scol_sb, scol_ps)
                binb_scaled[s] = wpool.tile([K, C], F32, tag="binbs",
                                            name=f"binbs{s}")
                nc.vector.tensor_scalar(binb_scaled[s], binb_sb, scol_sb, None,
                                        op0=ALU.mult)

            def drain_colsums(upto=None):
                while cs_pend and (upto is None or len(cs_pend) > upto):
                    sq = cs_pend.pop(0)
                    emit_colsum(*sq)
                    if sq[1] == NQ - 1:
                        emit_s_tail(sq[0])

            for it in range(T):
                for s in range(SPC):
                    _mark(nc, f'it{it}_Ahead_s{s}')
                    if it == 0:
                        # iter-0 folded weights are host-precomputed
                        m2t_sb[s] = m2t0_sb
                        eb_b[s] = ebb0_sb
                        continue
                    # A-head
                    m2t_ps = scpool.tile([128, CC, K], F32, tag=f"sc{s}",
                                         name=f"m2t_ps{s}")
                    for cc in range(CC):
                        for oc in range(CC):
                            nc.tensor.matmul(
                                m2t_ps[:, cc, :],
                                w_sb[:, oc, ts(cc, 128)],
                                muT[s][:, oc, :],
                                start=(oc == 0), stop=(oc == CC - 1))
                    beta_ps = rowpool.tile([1, K], F32, tag="row",
                                           name=f"beta_ps{s}")
                    for oc in range(CC):
                        nc.tensor.matmul(beta_ps, bin_sb[:, oc, :], muT[s][:, oc, :],
                                         start=(oc == 0), stop=(oc == CC - 1))
                    m2t_sb[s] = wpool.tile([128, CC, K], F16, tag="m2t",
                                           name=f"m2t_sb{s}")
                    nc.scalar.copy(m2t_sb[s], m2t_ps)
                    beta_sb = wpool.tile([1, K], F16, tag="beta",
                                         name=f"beta_sb{s}")
                    nc.scalar.copy(beta_sb, beta_ps)
                    eb_row = wpool.tile([1, K], F16, tag="eb_row",
                                        name=f"eb_row{s}")
                    nc.scalar.activation(eb_row, beta_sb, AF.Exp)
                    eb_ps = rowpool.tile([128, K], F32, tag="row",
                                         name=f"eb_ps{s}")
                    nc.tensor.matmul(eb_ps, ones_row, eb_row, start=True, stop=True)
                    eb_b[s] = wpool.tile([128, K], F16, tag="eb_b",
                                         name=f"eb_b{s}")
                    nc.scalar.copy(eb_b[s], eb_ps)

                for s in range(SPC):
                    _mark(nc, f'it{it}_Abody_s{s}')
                    # A-body
                    Z[s] = wpool.tile([128, NT, K], F16, tag="Z", name=f"Z_{s}")
                    r[s] = wpool.tile([128, NT], F32, tag="r", name=f"r{s}")
                    rv[s] = wpool.tile([128, NT], F32, tag="rinv", name=f"rinv{s}")
                    if USE_COLSUM:
                        s_ps[s] = spool.tile([1, NTQ, K], F32, tag="ssum",
                                             name=f"s_ps{s}")
                    E = None
                    if not INPLACE_Z:
                        E = wpool.tile([128, NT, K], F16, tag="E", name=f"E{s}")
                    for q in range(NQ):
                        _mark(nc, f'it{it}_lg_s{s}_q{q}')
                        lg = lgpool.tile([128, NTQ, K], F32, tag="lg",
                                         name=f"lg{s}_{q}")
                        for t8 in range(NTQ):
                            t = q * NTQ + t8
                            for cc in range(CC):
                                nc.tensor.matmul(
                                    lg[:, t8, :],
                                    X[s][:, cc, ts(t, 128)],
                                    m2t_sb[s][:, cc, :],
                                    start=(cc == 0), stop=(cc == CC - 1))
                        if USE_COLSUM:
                            drain_colsums(upto=1)
                        Zq = Z[s][:, ts(q, NTQ), :]
                        tgt = Zq if INPLACE_Z else E[:, ts(q, NTQ), :]
                        nc.scalar.activation(tgt, lg, AF.Exp,
                                             bias=expbias, scale=1.0)
                        nc.vector.tensor_tensor(
                            out=tgt, in0=tgt,
                            in1=bass.AP(tensor=eb_b[s].tensor,
                                        offset=eb_b[s].offset,
                                        ap=[eb_b[s].ap[0], [0, NTQ], [1, K]]),
                            op=ALU.mult)
                        nc.vector.reduce_sum(r[s][:, ts(q, NTQ)], tgt, axis=AX.X)
                        rq = rv[s][:, ts(q, NTQ)]
                        nc.vector.reciprocal(rq, r[s][:, ts(q, NTQ)])
                        nc.vector.tensor_tensor(
                            out=Zq, in0=tgt,
                            in1=bass.AP(tensor=rv[s].tensor,
                                        offset=rq.offset,
                                        ap=[rq.ap[0], rq.ap[1], [0, K]]),
                            op=ALU.mult)
                        if USE_COLSUM:
                            cs_pend.append((s, q))

                for s in range(SPC):
                    _mark(nc, f'it{it}_Bmain_s{s}')
                    # B-main
                    if USE_COLSUM:
                        drain_colsums(upto=0 if s == SPC - 1 else 1)
                    if USE_GT:
                        # cc-outer: PSUM accumulation groups must be
                        # contiguous per zero-region (interleaving the four
                        # cc groups in one bank corrupts partial sums — the
                        # start flag zeroes region-granular). Group 0 already
                        # consumes XT tiles incrementally via subtile deps.
                        GT_ps = scpool.tile([128, CC, K], F32, tag=f"sc{s}",
                                            name=f"GT_ps{s}")
                        for cc in range(CC):
                            for t in range(NT):
                                nc.tensor.matmul(GT_ps[:, cc, :],
                                                 XT[s][:, t, ts(cc, 128)],
                                                 Z[s][:, t, :],
                                                 start=(t == 0), stop=(t == NT - 1))
                        GT_sb = wpool.tile([128, CC, K], F16, tag="GT",
                                           name=f"GT_sb{s}")
                        nc.scalar.copy(GT_sb, GT_ps)
                    else:
                        G_ps = scpool.tile([K, C], F32, tag=f"sc{s}",
                                           name=f"G_ps{s}")
                        for t in range(NT):
                            nc.tensor.matmul(G_ps, Z[s][:, t, :], XT[s][:, t, :],
                                             start=(t == 0), stop=(t == NT - 1))
                        G_sb = wpool.tile([K, C], F16, tag="G", name=f"G_sb{s}")
                        nc.vector.tensor_copy(G_sb, G_ps)
                        GT_ps2 = scpool.tile([128, CC, K], F16, tag=f"sc{s}",
                                             name=f"GT_ps{s}")
                        for cc in range(CC):
                            nc.tensor.transpose(GT_ps2[:, cc, :],
                                                G_sb[:, ts(cc, 128)],
                                                ident[0:K, 0:K])
                        GT_sb = wpool.tile([128, CC, K], F16, tag="GT",
                                           name=f"GT_sb{s}")
                        nc.scalar.copy(GT_sb, GT_ps2)
                    if not USE_COLSUM:
                        s_ps[s] = rowpool.tile([1, K], F32, tag="row",
                                               name=f"sps{s}")
                        for t in range(NT):
                            nc.tensor.matmul(s_ps[s], ones_col, Z[s][:, t, :],
                                             start=(t == 0), stop=(t == NT - 1))
                        s_sb = wpool.tile([1, K], F32, tag="s", name=f"s_sb{s}")
                        nc.scalar.copy(s_sb, s_ps[s])
                        scol_ps = rowpool.tile([K, 1], F32, tag="row",
                                               name=f"scol_ps{s}")
                        nc.tensor.transpose(scol_ps, s_sb, ident32[0:1, 0:1])
                        qinv[s] = wpool.tile([K, 1], F32, tag="qinv",
                                             name=f"qinv{s}")
                        nc.vector.tensor_scalar(qinv[s], scol_ps, 1e-12, None,
                                                op0=ALU.add)
                        nc.vector.reciprocal(qinv[s], qinv[s])
                    mu_ps = scpool.tile([K, C], F32, tag=f"sc{s}", name=f"mu_ps{s}")
                    for cc in range(CC):
                        nc.tensor.matmul(mu_ps, GT_sb[:, cc, :], wt_sb[:, cc, :],
                                         start=(cc == 0), stop=(cc == CC - 1))
                    mu1 = wpool.tile([K, C], F32, tag="mu1", name=f"mu1_{s}")
                    if USE_COLSUM:
                        nc.vector.tensor_tensor(out=mu1, in0=mu_ps,
                                                in1=binb_scaled[s], op=ALU.add)
                    else:
                        nc.vector.tensor_scalar(mu1, mu_ps, qinv[s], None,
                                                op0=ALU.mult)
                        nc.vector.tensor_tensor(out=mu1, in0=mu1, in1=binb_sb,
                                                op=ALU.add)
                    n2 = wpool.tile([K, 1], F32, tag="n2", name=f"n2_{s}")
                    if USE_TTR:
                        sqd = wpool.tile([K, C], F16, tag="sq", name=f"sq{s}")
                        nc.vector.tensor_tensor_reduce(
                            out=sqd, in0=mu1, in1=mu1, scale=1.0, scalar=0.0,
                            op0=ALU.mult, op1=ALU.add, accum_out=n2)
                    else:
                        sqd = wpool.tile([K, C], F32, tag="sq", name=f"sq{s}")
                        nc.vector.tensor_tensor(out=sqd, in0=mu1, in1=mu1,
                                                op=ALU.mult)
                        nc.vector.reduce_sum(n2, sqd, axis=AX.X)
                    yy = wpool.tile([K, 1], F32, tag="yy", name=f"yy{s}")
                    ti = wpool.tile([K, 1], mybir.dt.int32, tag="ti",
                                    name=f"ti{s}")
                    nc.vector.tensor_scalar(ti, n2.bitcast(mybir.dt.int32), 1,
                                            None, op0=ALU.logical_shift_right)
                    nc.vector.tensor_scalar(ti, ti, -1, None,
                                            op0=ALU.bitwise_xor)
                    nc.vector.tensor_scalar(yy.bitcast(mybir.dt.int32), ti,
                                            0x5f3759df + 1, None,
                                            op0=ALU.add)
                    tb = wpool.tile([K, 1], F32, tag="tb", name=f"tb{s}")
                    for _ in range(2):
                        nc.vector.tensor_tensor(out=tb, in0=yy, in1=yy,
                                                op=ALU.mult)
                        nc.vector.tensor_tensor(out=tb, in0=tb, in1=n2,
                                                op=ALU.mult)
                        nc.vector.tensor_scalar(tb, tb, -0.5, 1.5,
                                                op0=ALU.mult, op1=ALU.add)
                        nc.vector.tensor_tensor(out=yy, in0=yy, in1=tb,
                                                op=ALU.mult)
                    mu16[s] = wpool.tile([K, C], F16, tag="mu16",
                                         name=f"mu16_{s}")
                    nc.vector.tensor_scalar(mu16[s], mu1, yy, None,
                                            op0=ALU.mult)

                for s in range(SPC):
                    _mark(nc, f'it{it}_Btail_s{s}')
                    # B-tail
                    muT_ps = scpool.tile([128, CC, K], F16, tag=f"sc{s}",
                                         name=f"muT_ps{s}")
                    for cc in range(CC):
                        nc.tensor.transpose(muT_ps[:, cc, :],
                                            mu16[s][:, ts(cc, 128)],
                                            ident[0:K, 0:K])
                    muT_new = wpool.tile([128, CC, K], F16, tag="muT",
                                         name=f"muT_new{s}")
                    nc.scalar.copy(muT_new, muT_ps)
                    muT[s] = muT_new

            # output path
            m3s, ZT = [None] * SPC, [None] * SPC
            for s in range(SPC):
                _mark(nc, f'out_zt_s{s}')
                m3_ps = scpool.tile([K, C], F32, tag=f"sc{s}", name=f"m3_ps{s}")
                for cc in range(CC):
                    nc.tensor.matmul(m3_ps, muT[s][:, cc, :], wot_sb[:, cc, :],
                                     start=(cc == 0), stop=(cc == CC - 1))
                m3s[s] = wpool.tile([K + 1, C], F16, tag="m3s", name=f"m3s{s}")
                nc.scalar.copy(m3s[s][0:K, :], m3_ps)
                nc.sync.dma_start(out=m3s[s][K:K + 1, :], in_=srow16)
                ZT[s] = wpool.tile([K + 1, N], F16, tag="ZT", name=f"ZT{s}")
                if ONES_DMA:
                    nc.sync.dma_start(out=ZT[s][K:K + 1, :], in_=ones16)
                else:
                    nc.vector.memset(ZT[s][K:K + 1, :], 1.0)
                for g in range(NT // 4):
                    zt_ps = rowpool.tile([K, 4, 128], F16, tag="row",
                                         name=f"zt_ps{s}_{g}")
                    for j in range(4):
                        nc.tensor.transpose(zt_ps[:, j, :], Z[s][:, g * 4 + j, :],
                                            ident)
                    dst = ZT[s][0:K, ts(g, 512)].rearrange("p (a b) -> p a b", a=4)
                    nc.scalar.copy(dst, zt_ps)

            # out2: o2 PSUM tiles rotate through 5 banks (lg pool 3 + the two
            # freed sc banks); osb staged into one [128, N] tile per (s, oc)
            # so the output DMA is a single trigger per (s, oc). The residual
            # x (fp16) streams in late as per-(s,oc) chunks; adds run on DVE
            # (fused with the PSUM->SBUF move) or gpsimd (after an ACT copy),
            # keeping the PE free of identity matmuls.
            odt = F16 if OUT_F16 else F32
            ocnt = 0
            for s in range(SPC):
                _mark(nc, f'out2_s{s}')
                for oc in range(CC):
                    xrc = XR[s][:, oc, :]
                    osb = opool.tile([128, N], odt, tag="osb",
                                     name=f"osb{s}_{oc}", bufs=2)
                    for nk in range(NK):
                        if ocnt % 5 < 3:
                            o2 = lgpool.tile([128, 512], F32, tag="lg",
                                             name=f"o2_{s}_{oc}_{nk}")
                        else:
                            o2 = scpool.tile([128, 512], F32,
                                             tag=f"sc{ocnt % 2}",
                                             name=f"o2_{s}_{oc}_{nk}")
                        ocnt += 1
                        xin = xrc[:, ts(nk, 512)]
                        dst = osb[:, ts(nk, 512)]
                        cls = nk if SPLIT_RES else (0 if (oc + nk) % 2 else 1)
                        if cls in (1, 2):
                            # residual on PE (identity matmul), copy on ACT
                            nc.tensor.matmul(o2, m3s[s][:, ts(oc, 128)],
                                             ZT[s][:, ts(nk, 512)],
                                             start=True, stop=False)
                            nc.tensor.matmul(o2, ident, xin,
                                             start=False, stop=True)
                            nc.scalar.copy(dst, o2)
                        elif cls == 0:
                            # ACT moves PSUM->SBUF, gpsimd adds x in SBUF
                            nc.tensor.matmul(o2, m3s[s][:, ts(oc, 128)],
                                             ZT[s][:, ts(nk, 512)],
                                             start=True, stop=True)
                            nc.scalar.copy(dst, o2)
                            nc.gpsimd.tensor_tensor(out=dst, in0=dst,
                                                    in1=xin, op=ALU.add)
                        else:
                            # DVE fused move+add
                            nc.tensor.matmul(o2, m3s[s][:, ts(oc, 128)],
                                             ZT[s][:, ts(nk, 512)],
                                             start=True, stop=True)
                            nc.vector.tensor_tensor(out=dst, in0=o2,
                                                    in1=xin, op=ALU.add)
                    nc.sync.dma_start(out=outp[s, oc], in_=osb)

    nc.compile()
    return nc


_NC_CACHE = None
_RUN_KWARGS: dict = {}
_LAST_RESULTS = None


def _get_nc():
    global _NC_CACHE
    if _NC_CACHE is None:
        _NC_CACHE = build_bass()
    return _NC_CACHE


def build_in_maps(x, w_in, b_in, w_out, b_out, gamma, beta, running_mean,
                  running_var, bases):
    x = np.asarray(x, np.float32)
    w_in = np.asarray(w_in, np.float32)
    b_in = np.asarray(b_in, np.float32)
    w_out = np.asarray(w_out, np.float32)
    b_out = np.asarray(b_out, np.float32)
    gamma = np.asarray(gamma, np.float32)
    beta = np.asarray(beta, np.float32)
    running_mean = np.asarray(running_mean, np.float32)
    running_var = np.asarray(running_var, np.float32)
    bases = np.asarray(bases, np.float32)

    inv = gamma / np.sqrt(running_var + BN_EPS)
    S = b_out * inv + beta - running_mean * inv
    wot = (w_out * inv[:, None]).T

    import ml_dtypes
    xr = x.reshape(B, C, N)
    x8 = np.ascontiguousarray(
        xr.reshape(B, CC, 128, N).transpose(0, 2, 1, 3)).astype(ml_dtypes.float8_e4m3)
    xresh = np.ascontiguousarray(xr.reshape(B, CC, 128, N)).astype(np.float16)
    xt8 = np.ascontiguousarray(
        xr.transpose(0, 2, 1).reshape(B, NT, 128, C).transpose(0, 2, 1, 3)
    ).astype(ml_dtypes.float8_e4m3)

    # iter-0 folded weights (mu0 = bases, unnormalized)
    m2t0 = (w_in.T.astype(np.float16).astype(np.float32)
            @ bases.T.astype(np.float16).astype(np.float32))      # (C, K)
    ebb0 = np.broadcast_to(
        np.exp(b_in @ bases.T.astype(np.float16).astype(np.float32))[None, :],
        (128, K))

    chunk = lambda a, f: a.reshape(CC, 128, f).transpose(1, 0, 2)
    wcat0 = np.ascontiguousarray(np.concatenate([
        chunk(np.ascontiguousarray(m2t0), K), chunk(b_in, 1),
    ], axis=2)).astype(np.float16)
    wcat1 = np.ascontiguousarray(np.concatenate([
        chunk(w_in, C),
        chunk(np.ascontiguousarray(w_in.T), C),
        chunk(np.ascontiguousarray(wot), C),
    ], axis=2)).astype(np.float16)
    srow16 = S.reshape(1, C).astype(np.float16)
    binrow = b_in.reshape(1, C).astype(np.float32)
    ones16 = np.ones((1, N), np.float16)
    ebb0 = np.ascontiguousarray(ebb0).astype(np.float16)

    in_maps = []
    for core in range(NCORES):
        sl = slice(core * SPC, (core + 1) * SPC)
        in_maps.append({
            "x8": x8[sl], "xres": xresh[sl], "xt8": xt8[sl],
            "wcat0": wcat0, "wcat1": wcat1, "ebb0": ebb0,
            "srow16": srow16, "binrow": binrow, "ones16": ones16,
        })
    return in_maps


def kernel(x, w_in, b_in, w_out, b_out, gamma, beta, running_mean, running_var,
           bases):
    in_maps = build_in_maps(x, w_in, b_in, w_out, b_out, gamma, beta,
                            running_mean, running_var, bases)
    nc = _get_nc()
    res = bass_utils.run_bass_kernel_spmd(nc, in_maps, core_ids=list(range(NCORES)),
                                          **_RUN_KWARGS)
    global _LAST_RESULTS
    _LAST_RESULTS = res
    out = np.empty((B, C, N), np.float32)
    for core in range(NCORES):
        o = res.results[core]["outp"]
        out[core * SPC:(core + 1) * SPC] = o.astype(np.float32).reshape(SPC, C, N)
    return out.reshape(B, C, H, W)
